# revision 1
# baseline (speedup 1.0000x reference)
"""Trainium2 Bass kernel for nn_KVCacheMemory (dual-attention memory gate).

Data-parallel over batch: each of the 8 NeuronCores computes one batch's two
single-head SxS attentions (S=4096, D=192) plus the flag-gated combine.

Per-core dataflow (all contractions ride the TensorEngine; no on-device
transposes, no vector reductions):
  - scoresT[k,q] = kT.T @ qT computed directly in the transposed layout so the
    exp() output (ACT, scale=1/sqrt(D) folded in) is already the moving
    operand of the oT accumulation matmul.
  - A ones-column appended to v makes the softmax row-sum fall out of the oT
    matmul as an extra row; a unit column appended to Wo carries that row-sum
    through the output projection, so it lands as column 192 of the final
    [128,193] PSUM tile, per-partition aligned for one reciprocal + fused
    scalar multiply (softmax normalization commutes with the linear Wo).
"""
import numpy as np
import ml_dtypes

import concourse.bacc as bacc
import concourse.tile as tile
import concourse.mybir as mybir
from concourse.bass_utils import run_bass_kernel_spmd

B, S, D = 8, 4096, 192
MEM_READ, MEM_WRITE, MEM_READY = 156, 157, 158
P = 128          # partitions / tile rows
QB = 512         # q block (matmul free dim / PSUM bank)
NQB = S // QB    # 8
KC = 128         # key chunk (contraction tile)
NKC = S // KC    # 32
NT = S // P      # 32 row tiles
D0, D1 = 128, 64  # contraction split of D=192
SCALE = 1.0 / float(np.sqrt(D))
F32 = mybir.dt.float32
BF16 = mybir.dt.bfloat16
FP8 = mybir.dt.float8e4
DR = mybir.MatmulPerfMode.DoubleRow
VBLK = 208       # v_ext block stride (16B-aligned for DoubleRow lhsT step)
N_CORES = 8

_CACHE = {}


def _build():
    nc = bacc.Bacc("TRN2", target_bir_lowering=False, debug=False,
                   num_devices=N_CORES)
    x = nc.dram_tensor("x", [S, D], F32, kind="ExternalInput").ap()
    xt0 = nc.dram_tensor("xt0", [D0, S], BF16, kind="ExternalInput").ap()
    xt1 = nc.dram_tensor("xt1", [D1, S], BF16, kind="ExternalInput").ap()
    wqkv0 = nc.dram_tensor("wqkv0", [D0, 6 * D], BF16, kind="ExternalInput").ap()
    wqkv1 = nc.dram_tensor("wqkv1", [D1, 6 * D], BF16, kind="ExternalInput").ap()
    woe0 = nc.dram_tensor("woe0", [D0, 2 * (D + 1)], BF16, kind="ExternalInput").ap()
    woe1 = nc.dram_tensor("woe1", [D1 + 1, 2 * (D + 1)], BF16, kind="ExternalInput").ap()
    params = nc.dram_tensor("params", [P, 4], F32, kind="ExternalInput").ap()
    out = nc.dram_tensor("out", [S, D], F32, kind="ExternalOutput").ap()

    with tile.TileContext(nc) as tc:
        _emit(nc, tc, x, xt0, xt1, wqkv0, wqkv1, woe0, woe1, params, out)
    nc.compile()
    return nc


def _emit(nc, tc, x, xt0, xt1, wqkv0, wqkv1, woe0, woe1, params, out):
    from contextlib import ExitStack
    with ExitStack() as st:
        cpool = st.enter_context(tc.tile_pool(name="const", bufs=1))
        bigpool = st.enter_context(tc.tile_pool(name="big", bufs=1))
        apool = st.enter_context(tc.tile_pool(name="attn", bufs=6))
        opool = st.enter_context(tc.tile_pool(name="osb", bufs=2))
        xpool = st.enter_context(tc.tile_pool(name="xin", bufs=3))
        tpool = st.enter_context(tc.tile_pool(name="tmp", bufs=3))
        # PSUM budget (8 banks): mm 3x[128,1024]=6, oT0+oT1 1x each=2;
        # res tiles rotate through the oT0 slot (tag-shared, freed post-copy)
        mmpool = st.enter_context(tc.tile_pool(name="mm", bufs=3, space="PSUM"))
        oaccpool = st.enter_context(tc.tile_pool(name="oacc", bufs=1, space="PSUM"))

        # resident constants / activations. Weights + params first (small,
        # gate everything); xt loads chunked so phase-A chunk ci only waits
        # for its own slice.
        pp = cpool.tile([P, 4], F32, tag="pp")
        nc.sync.dma_start(pp, params)
        w0s = cpool.tile([D0, 6 * D], BF16, tag="w0s")
        nc.sync.dma_start(w0s, wqkv0)
        w1s = cpool.tile([D1, 6 * D], BF16, tag="w1s")
        nc.sync.dma_start(w1s, wqkv1)
        xt0s = cpool.tile([D0, S], BF16, tag="xt0s")
        xt1s = cpool.tile([D1, S], BF16, tag="xt1s")
        for sb in range(NQB):
            sl = slice(sb * QB, (sb + 1) * QB)
            nc.sync.dma_start(xt0s[:, sl], xt0[:, sl])
            nc.sync.dma_start(xt1s[:, sl], xt1[:, sl])
        woe0s = cpool.tile([D0, 2 * (D + 1)], BF16, tag="woe0s")
        nc.sync.dma_start(woe0s, woe0)
        woe1s = cpool.tile([D1 + 1, 2 * (D + 1)], BF16, tag="woe1s")
        nc.sync.dma_start(woe1s, woe1)
        # pre-fault the exp ACT table so the ~2.7us load overlaps input DMAs
        warm = cpool.tile([1, 1], F32, tag="warm")
        nc.scalar.activation(warm, pp[0:1, 0:1],
                             mybir.ActivationFunctionType.Exp)

        # out accumulator [128, 32*192] f32 (tile g lives at cols g*192)
        out_acc = bigpool.tile([P, NT * D], F32, tag="out_acc")

        # per-attention persistent buffers (distinct tags so att1's phase A
        # can be emitted under att0's ACT-bound phase B)
        bufs = []
        for att in range(2):
            qTd = bigpool.tile([96, 2 * S], FP8, tag=f"qTd{att}", name="qTd")
            kTd = bigpool.tile([96, 2 * S], FP8, tag=f"kTd{att}", name="kTd")
            v_ext = bigpool.tile([P, NT * VBLK], FP8, tag=f"v_ext{att}",
                                 name="v_ext")
            bufs.append((qTd, kTd, v_ext))

        def phaseA_unit(att, ci, u):
            """Emit unit u (0..7) of phase-A chunk ci for `att`:
            u 0-1 = q halves, 2-3 = k halves, 4-7 = v tiles.
            qTd/kTd layout [96, 2, S] with e = 96*o + ki."""
            qTd, kTd, v_ext = bufs[att]
            qoff = (3 * att + 0) * D
            koff = (3 * att + 1) * D
            voff = (3 * att + 2) * D
            if ci == 0 and u == 0:
                ones = v_ext.rearrange("p (t c) -> p t c", c=VBLK)[:, :, D:D + 1]
                nc.vector.memset(ones, 1.0)
            sb = ci
            if u < 4:
                dst, woffp = ((qTd, qoff), (kTd, koff))[u // 2]
                half = u % 2
                mlo = 96 * half
                ps = mmpool.tile([P, QB], F32, tag="mm", name="ps_proj")
                nc.tensor.matmul(
                    ps[:96, :], w0s[:, woffp + mlo:woffp + mlo + 96],
                    xt0s[:, sb * QB:(sb + 1) * QB], start=True, stop=False)
                nc.tensor.matmul(
                    ps[:96, :], w1s[:, woffp + mlo:woffp + mlo + 96],
                    xt1s[:, sb * QB:(sb + 1) * QB], start=False, stop=True)
                nc.vector.tensor_copy(
                    dst[:, half * S + sb * QB:half * S + (sb + 1) * QB],
                    ps[:96, :])
            else:
                t = 4 * ci + (u - 4)
                ps = mmpool.tile([P, QB], F32, tag="mm", name="ps_v")
                nc.tensor.matmul(ps[:, :D], xt0s[:, t * P:(t + 1) * P],
                                 w0s[:, voff:voff + D], start=True, stop=False)
                nc.tensor.matmul(ps[:, :D], xt1s[:, t * P:(t + 1) * P],
                                 w1s[:, voff:voff + D], start=False, stop=True)
                nc.vector.tensor_copy(
                    v_ext[:, t * VBLK:t * VBLK + D], ps[:, :D])

        def phaseA_chunk(att, ci):
            for u in range(8):
                phaseA_unit(att, ci, u)

        NPR = NKC // 2
        ostate = {}

        def phaseB_main(att, qb, interleave=None):
            qTd, kTd, v_ext = bufs[att]
            kT3 = kTd.rearrange("p (o s) -> p o s", o=2)
            qT3 = qTd.rearrange("p (o s) -> p o s", o=2)
            ve3 = v_ext.rearrange("p (t c) -> p t c", c=VBLK)
            oT0 = oaccpool.tile([P, QB], F32, tag="oT0")
            oT1 = oaccpool.tile([D1 + 1, QB], F32, tag="oT1")
            ostate[(att, qb)] = (oT0, oT1)
            qs3 = qT3[:, :, qb * QB:(qb + 1) * QB]
            for pr in range(NPR):
                # two key-chunks' scoresT side by side in one 2-bank tile
                sc = mmpool.tile([P, 2 * QB], F32, tag="mm", name="sc")
                for h in range(2):
                    kc = 2 * pr + h
                    nc.tensor.matmul(sc[:, h * QB:(h + 1) * QB],
                                     kT3[:, :, kc * KC:(kc + 1) * KC],
                                     qs3, start=True, stop=True,
                                     perf_mode=DR)
                at = apool.tile([P, 2 * QB], FP8, tag="at")
                nc.scalar.activation(at, sc, mybir.ActivationFunctionType.Exp,
                                     scale=SCALE)
                at3 = at.rearrange("p (o n) -> p o n", o=2)
                nc.tensor.matmul(oT0, ve3[:, 2 * pr:2 * pr + 2, 0:D0], at3,
                                 start=(pr == 0), stop=(pr == NPR - 1),
                                 perf_mode=DR)
                nc.tensor.matmul(oT1, ve3[:, 2 * pr:2 * pr + 2, D0:D + 1],
                                 at3, start=(pr == 0), stop=(pr == NPR - 1),
                                 perf_mode=DR)
                if interleave is not None:
                    interleave(pr)

        def phaseB_epi(att, qb):
            wo_off = att * (D + 1)
            flag_col = 1 + att
            oT0, oT1 = ostate.pop((att, qb))
            if True:
                oT0s = opool.tile([P, QB], BF16, tag="oT0s")
                nc.vector.tensor_copy(oT0s, oT0)
                oT1s = opool.tile([D1 + 1, QB], BF16, tag="oT1s")
                nc.vector.tensor_copy(oT1s, oT1)

                for qt in range(4):
                    g = qb * 4 + qt
                    res = oaccpool.tile([P, QB], F32, tag="oT0", name="res")
                    res = res[:, 0:D + 1]
                    nc.tensor.matmul(res, oT0s[:, qt * P:(qt + 1) * P],
                                     woe0s[:, wo_off:wo_off + D + 1],
                                     start=True, stop=False)
                    nc.tensor.matmul(res, oT1s[:, qt * P:(qt + 1) * P],
                                     woe1s[:, wo_off:wo_off + D + 1],
                                     start=False, stop=True)
                    rec = tpool.tile([P, 1], F32, tag="rec")
                    nc.vector.reciprocal(rec, res[:, D:D + 1])
                    tmp = tpool.tile([P, D], F32, tag="tmp")
                    nc.vector.tensor_scalar(
                        tmp, res[:, 0:D], rec, pp[:, flag_col:flag_col + 1],
                        op0=mybir.AluOpType.mult, op1=mybir.AluOpType.mult)
                    acc = out_acc[:, g * D:(g + 1) * D]
                    if att == 0:
                        xt = xpool.tile([P, D], F32, tag="xt")
                        nc.sync.dma_start(xt, x[g * P:(g + 1) * P, :])
                        nc.vector.tensor_scalar(
                            acc, xt, pp[:, 0:1], None, op0=mybir.AluOpType.mult)
                        nc.vector.tensor_add(acc, acc, tmp)
                    else:
                        nc.vector.tensor_add(acc, acc, tmp)
                        nc.vector.memset(acc[:, MEM_READ:MEM_WRITE + 1], 0.0)
                        nc.vector.tensor_copy(acc[:, MEM_READY:MEM_READY + 1],
                                              pp[:, 3:4])
                        nc.sync.dma_start(out[g * P:(g + 1) * P, :], acc)

        # driver: A(0) units feed B(0,qb0) pair-by-pair (chunk ci complete
        # by pair 2ci); epilogues deferred one qb so the next qb's score
        # matmuls keep ACT fed; A(1) units spread across B(0)'s qb loops.
        def ilv0(pr):
            # head: only k/v units of att0 chunk ci = pr//2+1 (q for block
            # ci isn't needed until B(0,ci)); chunk1's q at pairs 14/15
            ci = pr // 2 + 1
            if ci < NQB:
                for u in ((2, 3, 4) if pr % 2 == 0 else (5, 6, 7)):
                    phaseA_unit(0, ci, u)
            elif pr in (14, 15):
                phaseA_unit(0, 1, pr - 14)

        def ilv_b0(qb):
            # under B(0,qb): att1 chunk qb-1 units at even pairs, att0
            # chunk qb+1's q units at pairs 1/3
            def f(pr):
                if pr % 2 == 0:
                    phaseA_unit(1, qb - 1, pr // 2)
                elif pr in (1, 3) and qb + 1 < NQB:
                    phaseA_unit(0, qb + 1, (pr - 1) // 2)
            return f

        def ilv_a1_last(pr):
            if pr % 2 == 0:
                phaseA_unit(1, NQB - 1, pr // 2)

        phaseA_chunk(0, 0)
        phaseB_main(0, 0, interleave=ilv0)
        for qb in range(1, NQB):
            phaseB_main(0, qb, interleave=ilv_b0(qb))
            phaseB_epi(0, qb - 1)
        for qb in range(NQB):
            # A(1) chunk 7 rides under B(1,0)'s first pairs
            ilv = ilv_a1_last if qb == 0 else None
            phaseB_main(1, qb, interleave=ilv)
            phaseB_epi(0 if qb == 0 else 1, NQB - 1 if qb == 0 else qb - 1)
        phaseB_epi(1, NQB - 1)


def _prep_core_inputs(x_full, weights):
    """Host-side shard/layout prep. weights: dict of the 8 [192,192] f32."""
    bf = ml_dtypes.bfloat16
    worder = ["Wq_r", "Wk_r", "Wv_r", "Wq_w", "Wk_w", "Wv_w"]
    wcat = np.concatenate([np.ascontiguousarray(weights[n].T) for n in worder],
                          axis=1).astype(bf)  # [192, 6*192]
    woe = np.zeros((D + 1, 2 * (D + 1)), np.float32)
    for a, n in enumerate(("Wo_r", "Wo_w")):
        woe[:D, a * (D + 1):a * (D + 1) + D] = weights[n].T
        woe[D, a * (D + 1) + D] = 1.0
    woe = woe.astype(bf)
    in_maps = []
    for c in range(N_CORES):
        xb = np.ascontiguousarray(x_full[c]).astype(np.float32)  # [4096,192]
        xT = np.ascontiguousarray(xb.T).astype(bf)               # [192,4096]
        rg = float(xb[0, MEM_READ])
        wg = float(xb[0, MEM_WRITE])
        pvec = np.array([1.0 - rg - wg, rg, wg, rg + wg], np.float32)
        in_maps.append({
            "x": xb,
            "xt0": np.ascontiguousarray(xT[:D0]),
            "xt1": np.ascontiguousarray(xT[D0:]),
            "wqkv0": np.ascontiguousarray(wcat[:D0]),
            "wqkv1": np.ascontiguousarray(wcat[D0:]),
            "woe0": np.ascontiguousarray(woe[:D0]),
            "woe1": np.ascontiguousarray(woe[D0:]),
            "params": np.tile(pvec, (P, 1)),
        })
    return in_maps


def _run(inputs, **spmd_kwargs):
    if "nc" not in _CACHE:
        _CACHE["nc"] = _build()
    nc = _CACHE["nc"]
    x_full = np.asarray(inputs["x"], np.float32)
    weights = {k: np.asarray(inputs[k], np.float32) for k in
               ("Wq_r", "Wk_r", "Wv_r", "Wo_r", "Wq_w", "Wk_w", "Wv_w", "Wo_w")}
    in_maps = _prep_core_inputs(x_full, weights)
    res = run_bass_kernel_spmd(nc, in_maps, list(range(N_CORES)), **spmd_kwargs)
    out = np.stack([res.results[c]["out"] for c in range(N_CORES)], axis=0)
    return out.astype(np.float32), res


def kernel(**inputs):
    out, _ = _run(inputs)
    return out


def kernel_traced(**inputs):
    """For test.py: also returns BassKernelResults with profile info."""
    return _run(inputs, trace=True)



# revision 2
# speedup vs baseline: 1.1912x; 1.1912x over previous
"""Trainium2 Bass kernel for nn_KVCacheMemory (dual-attention memory gate).

Data-parallel over batch: each of the 8 NeuronCores computes one batch's two
single-head SxS attentions (S=4096, D=192) plus the flag-gated combine.

Per-core dataflow (all contractions ride the TensorEngine; no on-device
transposes, no vector reductions):
  - scoresT[k,q] = kT.T @ qT computed directly in the transposed layout so the
    exp() output (ACT, scale=1/sqrt(D) folded in) is already the moving
    operand of the oT accumulation matmul.
  - A ones-column appended to v makes the softmax row-sum fall out of the oT
    matmul as an extra row; a unit column appended to Wo carries that row-sum
    through the output projection, so it lands as column 192 of the final
    [128,193] PSUM tile, per-partition aligned for one reciprocal + fused
    scalar multiply (softmax normalization commutes with the linear Wo).
"""
import numpy as np
import ml_dtypes

import concourse.bacc as bacc
import concourse.tile as tile
import concourse.mybir as mybir
import concourse.bass_utils as _bu
from concourse.bass_utils import run_bass_kernel_spmd

# Extra walrus flags (e.g. --enable-double-pixel-opt) appended via env knob.
if not getattr(_bu.get_walrus_args, "_extra_patched", False):
    _orig_gwa = _bu.get_walrus_args

    def _gwa(*a, **kw):
        import os as _os
        args = _orig_gwa(*a, **kw)
        extra = _os.environ.get("BASSK_WALRUS_EXTRA", "")
        if extra:
            args = list(args) + extra.split()
        return args

    _gwa._extra_patched = True
    _bu.get_walrus_args = _gwa

B, S, D = 8, 4096, 192
MEM_READ, MEM_WRITE, MEM_READY = 156, 157, 158
P = 128          # partitions / tile rows
QB = 512         # q block (matmul free dim / PSUM bank)
NQB = S // QB    # 8
KC = 128         # key chunk (contraction tile)
NKC = S // KC    # 32
NT = S // P      # 32 row tiles
D0, D1 = 128, 64  # contraction split of D=192
SCALE = 1.0 / float(np.sqrt(D))
F32 = mybir.dt.float32
BF16 = mybir.dt.bfloat16
FP8 = mybir.dt.float8e4
DR = mybir.MatmulPerfMode.DoubleRow
VBLK = 208       # v_ext block stride (16B-aligned for DoubleRow lhsT step)
N_CORES = 8

_CACHE = {}


def _build():
    nc = bacc.Bacc("TRN2", target_bir_lowering=False, debug=False,
                   num_devices=N_CORES)
    x = nc.dram_tensor("x", [S, D], F32, kind="ExternalInput").ap()
    xt0 = nc.dram_tensor("xt0", [D0, S], BF16, kind="ExternalInput").ap()
    xt1 = nc.dram_tensor("xt1", [D1, S], BF16, kind="ExternalInput").ap()
    wqkv0 = nc.dram_tensor("wqkv0", [D0, 6 * D], BF16, kind="ExternalInput").ap()
    wqkv1 = nc.dram_tensor("wqkv1", [D1, 6 * D], BF16, kind="ExternalInput").ap()
    woe0 = nc.dram_tensor("woe0", [D0, 2 * (D + 1)], BF16, kind="ExternalInput").ap()
    woe1 = nc.dram_tensor("woe1", [D1 + 1, 2 * (D + 1)], BF16, kind="ExternalInput").ap()
    params = nc.dram_tensor("params", [P, 4], F32, kind="ExternalInput").ap()
    out = nc.dram_tensor("out", [S, D], F32, kind="ExternalOutput").ap()

    with tile.TileContext(nc) as tc:
        _emit(nc, tc, x, xt0, xt1, wqkv0, wqkv1, woe0, woe1, params, out)
    nc.compile()
    return nc


def _emit(nc, tc, x, xt0, xt1, wqkv0, wqkv1, woe0, woe1, params, out):
    from contextlib import ExitStack
    with ExitStack() as st:
        cpool = st.enter_context(tc.tile_pool(name="const", bufs=1))
        bigpool = st.enter_context(tc.tile_pool(name="big", bufs=1))
        apool = st.enter_context(tc.tile_pool(name="attn", bufs=6))
        opool = st.enter_context(tc.tile_pool(name="osb", bufs=2))
        xpool = st.enter_context(tc.tile_pool(name="xin", bufs=3))
        tpool = st.enter_context(tc.tile_pool(name="tmp", bufs=3))
        # PSUM budget (8 banks): mm 3x[128,1024]=6, oT0+oT1 1x each=2;
        # res tiles rotate through the oT0 slot (tag-shared, freed post-copy)
        mmpool = st.enter_context(tc.tile_pool(name="mm", bufs=3, space="PSUM"))
        oaccpool = st.enter_context(tc.tile_pool(name="oacc", bufs=1, space="PSUM"))

        # resident constants / activations. Weights + params first (small,
        # gate everything); xt loads chunked so phase-A chunk ci only waits
        # for its own slice.
        pp = cpool.tile([P, 4], F32, tag="pp")
        nc.sync.dma_start(pp, params)
        w0s = cpool.tile([D0, 6 * D], BF16, tag="w0s")
        nc.sync.dma_start(w0s, wqkv0)
        w1s = cpool.tile([D1, 6 * D], BF16, tag="w1s")
        nc.sync.dma_start(w1s, wqkv1)
        xt0s = cpool.tile([D0, S], BF16, tag="xt0s")
        xt1s = cpool.tile([D1, S], BF16, tag="xt1s")
        for sb in range(NQB):
            sl = slice(sb * QB, (sb + 1) * QB)
            nc.sync.dma_start(xt0s[:, sl], xt0[:, sl])
            nc.sync.dma_start(xt1s[:, sl], xt1[:, sl])
        woe0s = cpool.tile([D0, 2 * (D + 1)], BF16, tag="woe0s")
        nc.sync.dma_start(woe0s, woe0)
        woe1s = cpool.tile([D1 + 1, 2 * (D + 1)], BF16, tag="woe1s")
        nc.sync.dma_start(woe1s, woe1)
        # pre-fault the exp ACT table so the ~2.7us load overlaps input DMAs
        warm = cpool.tile([1, 1], F32, tag="warm")
        nc.scalar.activation(warm, pp[0:1, 0:1],
                             mybir.ActivationFunctionType.Exp)

        # out accumulator [128, 32*192] f32 (tile g lives at cols g*192)
        out_acc = bigpool.tile([P, NT * D], F32, tag="out_acc")

        # per-attention persistent buffers (distinct tags so att1's phase A
        # can be emitted under att0's ACT-bound phase B)
        bufs = []
        for att in range(2):
            qTd = bigpool.tile([96, 2 * S], FP8, tag=f"qTd{att}", name="qTd")
            kTd = bigpool.tile([96, 2 * S], FP8, tag=f"kTd{att}", name="kTd")
            v_ext = bigpool.tile([P, NT * VBLK], FP8, tag=f"v_ext{att}",
                                 name="v_ext")
            bufs.append((qTd, kTd, v_ext))

        def phaseA_unit(att, ci, u):
            """Emit unit u (0..7) of phase-A chunk ci for `att`:
            u 0-1 = q halves, 2-3 = k halves, 4-7 = v tiles.
            qTd/kTd layout [96, 2, S] with e = 96*o + ki."""
            qTd, kTd, v_ext = bufs[att]
            qoff = (3 * att + 0) * D
            koff = (3 * att + 1) * D
            voff = (3 * att + 2) * D
            if ci == 0 and u == 0:
                ones = v_ext.rearrange("p (t c) -> p t c", c=VBLK)[:, :, D:D + 1]
                nc.vector.memset(ones, 1.0)
            sb = ci
            if u < 4:
                dst, woffp = ((qTd, qoff), (kTd, koff))[u // 2]
                half = u % 2
                mlo = 96 * half
                ps = mmpool.tile([P, QB], F32, tag="mm", name="ps_proj")
                nc.tensor.matmul(
                    ps[:96, :], w0s[:, woffp + mlo:woffp + mlo + 96],
                    xt0s[:, sb * QB:(sb + 1) * QB], start=True, stop=False)
                nc.tensor.matmul(
                    ps[:96, :], w1s[:, woffp + mlo:woffp + mlo + 96],
                    xt1s[:, sb * QB:(sb + 1) * QB], start=False, stop=True)
                nc.vector.tensor_copy(
                    dst[:, half * S + sb * QB:half * S + (sb + 1) * QB],
                    ps[:96, :])
            else:
                t = 4 * ci + (u - 4)
                ps = mmpool.tile([P, QB], F32, tag="mm", name="ps_v")
                nc.tensor.matmul(ps[:, :D], xt0s[:, t * P:(t + 1) * P],
                                 w0s[:, voff:voff + D], start=True, stop=False)
                nc.tensor.matmul(ps[:, :D], xt1s[:, t * P:(t + 1) * P],
                                 w1s[:, voff:voff + D], start=False, stop=True)
                nc.vector.tensor_copy(
                    v_ext[:, t * VBLK:t * VBLK + D], ps[:, :D])

        def phaseA_chunk(att, ci):
            for u in range(8):
                phaseA_unit(att, ci, u)

        NPR = NKC // 2
        ostate = {}

        def phaseB_main(att, qb, interleave=None):
            qTd, kTd, v_ext = bufs[att]
            kT3 = kTd.rearrange("p (o s) -> p o s", o=2)
            qT3 = qTd.rearrange("p (o s) -> p o s", o=2)
            ve3 = v_ext.rearrange("p (t c) -> p t c", c=VBLK)
            oT0 = oaccpool.tile([P, QB], F32, tag="oT0")
            oT1 = oaccpool.tile([D1 + 1, QB], F32, tag="oT1")
            ostate[(att, qb)] = (oT0, oT1)
            qs3 = qT3[:, :, qb * QB:(qb + 1) * QB]
            for pr in range(NPR):
                # two key-chunks' scoresT side by side in one 2-bank tile
                sc = mmpool.tile([P, 2 * QB], F32, tag="mm", name="sc")
                for h in range(2):
                    kc = 2 * pr + h
                    nc.tensor.matmul(sc[:, h * QB:(h + 1) * QB],
                                     kT3[:, :, kc * KC:(kc + 1) * KC],
                                     qs3, start=True, stop=True,
                                     perf_mode=DR)
                at = apool.tile([P, 2 * QB], FP8, tag="at")
                nc.scalar.activation(at, sc, mybir.ActivationFunctionType.Exp,
                                     scale=SCALE)
                at3 = at.rearrange("p (o n) -> p o n", o=2)
                nc.tensor.matmul(oT0, ve3[:, 2 * pr:2 * pr + 2, 0:D0], at3,
                                 start=(pr == 0), stop=(pr == NPR - 1),
                                 perf_mode=DR)
                nc.tensor.matmul(oT1, ve3[:, 2 * pr:2 * pr + 2, D0:D + 1],
                                 at3, start=(pr == 0), stop=(pr == NPR - 1),
                                 perf_mode=DR)
                if interleave is not None:
                    interleave(pr)

        def phaseB_epi(att, qb):
            wo_off = att * (D + 1)
            flag_col = 1 + att
            oT0, oT1 = ostate.pop((att, qb))
            if True:
                oT0s = opool.tile([P, QB], BF16, tag="oT0s")
                nc.vector.tensor_copy(oT0s, oT0)
                oT1s = opool.tile([D1 + 1, QB], BF16, tag="oT1s")
                nc.vector.tensor_copy(oT1s, oT1)

                for qt in range(4):
                    g = qb * 4 + qt
                    res = oaccpool.tile([P, QB], F32, tag="oT0", name="res")
                    res = res[:, 0:D + 1]
                    nc.tensor.matmul(res, oT0s[:, qt * P:(qt + 1) * P],
                                     woe0s[:, wo_off:wo_off + D + 1],
                                     start=True, stop=False)
                    nc.tensor.matmul(res, oT1s[:, qt * P:(qt + 1) * P],
                                     woe1s[:, wo_off:wo_off + D + 1],
                                     start=False, stop=True)
                    rec = tpool.tile([P, 1], F32, tag="rec")
                    nc.vector.reciprocal(rec, res[:, D:D + 1])
                    tmp = tpool.tile([P, D], F32, tag="tmp")
                    nc.vector.tensor_scalar(
                        tmp, res[:, 0:D], rec, pp[:, flag_col:flag_col + 1],
                        op0=mybir.AluOpType.mult, op1=mybir.AluOpType.mult)
                    acc = out_acc[:, g * D:(g + 1) * D]
                    if att == 0:
                        xt = xpool.tile([P, D], F32, tag="xt")
                        nc.sync.dma_start(xt, x[g * P:(g + 1) * P, :])
                        nc.vector.tensor_scalar(
                            acc, xt, pp[:, 0:1], None, op0=mybir.AluOpType.mult)
                        nc.vector.tensor_add(acc, acc, tmp)
                    else:
                        nc.vector.tensor_add(acc, acc, tmp)
                        nc.vector.memset(acc[:, MEM_READ:MEM_WRITE + 1], 0.0)
                        nc.vector.tensor_copy(acc[:, MEM_READY:MEM_READY + 1],
                                              pp[:, 3:4])
                        nc.sync.dma_start(out[g * P:(g + 1) * P, :], acc)

        # driver: A(0) units feed B(0,qb0) pair-by-pair (chunk ci complete
        # by pair 2ci); epilogues deferred one qb so the next qb's score
        # matmuls keep ACT fed; A(1) units spread across B(0)'s qb loops.
        def ilv0(pr):
            # head: only k/v units of att0 chunk ci = pr//2+1 (q for block
            # ci isn't needed until B(0,ci)); chunk1's q at pairs 14/15
            ci = pr // 2 + 1
            if ci < NQB:
                for u in ((2, 3, 4) if pr % 2 == 0 else (5, 6, 7)):
                    phaseA_unit(0, ci, u)
            elif pr in (14, 15):
                phaseA_unit(0, 1, pr - 14)

        def ilv_b0(qb):
            # under B(0,qb): att1 chunk qb-1 units at even pairs, att0
            # chunk qb+1's q units at pairs 1/3
            def f(pr):
                if pr % 2 == 0:
                    phaseA_unit(1, qb - 1, pr // 2)
                elif pr in (1, 3) and qb + 1 < NQB:
                    phaseA_unit(0, qb + 1, (pr - 1) // 2)
            return f

        def ilv_a1_last(pr):
            if pr % 2 == 0:
                phaseA_unit(1, NQB - 1, pr // 2)

        phaseA_chunk(0, 0)
        phaseB_main(0, 0, interleave=ilv0)
        for qb in range(1, NQB):
            phaseB_main(0, qb, interleave=ilv_b0(qb))
            phaseB_epi(0, qb - 1)
        for qb in range(NQB):
            # A(1) chunk 7 rides under B(1,0)'s first pairs
            ilv = ilv_a1_last if qb == 0 else None
            phaseB_main(1, qb, interleave=ilv)
            phaseB_epi(0 if qb == 0 else 1, NQB - 1 if qb == 0 else qb - 1)
        phaseB_epi(1, NQB - 1)


def _prep_core_inputs(x_full, weights):
    """Host-side shard/layout prep. weights: dict of the 8 [192,192] f32."""
    bf = ml_dtypes.bfloat16
    worder = ["Wq_r", "Wk_r", "Wv_r", "Wq_w", "Wk_w", "Wv_w"]
    wcat = np.concatenate([np.ascontiguousarray(weights[n].T) for n in worder],
                          axis=1).astype(bf)  # [192, 6*192]
    woe = np.zeros((D + 1, 2 * (D + 1)), np.float32)
    for a, n in enumerate(("Wo_r", "Wo_w")):
        woe[:D, a * (D + 1):a * (D + 1) + D] = weights[n].T
        woe[D, a * (D + 1) + D] = 1.0
    woe = woe.astype(bf)
    in_maps = []
    for c in range(N_CORES):
        xb = np.ascontiguousarray(x_full[c]).astype(np.float32)  # [4096,192]
        xT = np.ascontiguousarray(xb.T).astype(bf)               # [192,4096]
        rg = float(xb[0, MEM_READ])
        wg = float(xb[0, MEM_WRITE])
        pvec = np.array([1.0 - rg - wg, rg, wg, rg + wg], np.float32)
        in_maps.append({
            "x": xb,
            "xt0": np.ascontiguousarray(xT[:D0]),
            "xt1": np.ascontiguousarray(xT[D0:]),
            "wqkv0": np.ascontiguousarray(wcat[:D0]),
            "wqkv1": np.ascontiguousarray(wcat[D0:]),
            "woe0": np.ascontiguousarray(woe[:D0]),
            "woe1": np.ascontiguousarray(woe[D0:]),
            "params": np.tile(pvec, (P, 1)),
        })
    return in_maps


def _run(inputs, **spmd_kwargs):
    if "nc" not in _CACHE:
        _CACHE["nc"] = _build()
    nc = _CACHE["nc"]
    x_full = np.asarray(inputs["x"], np.float32)
    weights = {k: np.asarray(inputs[k], np.float32) for k in
               ("Wq_r", "Wk_r", "Wv_r", "Wo_r", "Wq_w", "Wk_w", "Wv_w", "Wo_w")}
    in_maps = _prep_core_inputs(x_full, weights)
    res = run_bass_kernel_spmd(nc, in_maps, list(range(N_CORES)), **spmd_kwargs)
    out = np.stack([res.results[c]["out"] for c in range(N_CORES)], axis=0)
    return out.astype(np.float32), res


def kernel(**inputs):
    out, _ = _run(inputs)
    return out


def kernel_traced(**inputs):
    """For test.py: also returns BassKernelResults with profile info."""
    return _run(inputs, trace=True)



# revision 7
# speedup vs baseline: 1.3258x; 1.1130x over previous
"""Trainium2 Bass kernel for nn_KVCacheMemory (dual-attention memory gate).

Data-parallel over batch: each of the 8 NeuronCores computes one batch's two
single-head SxS attentions (S=4096, D=192) plus the flag-gated combine.

Per-core dataflow (all contractions ride the TensorEngine; no on-device
transposes, no vector reductions). Everything fp8 DoubleRow (+ walrus
double-pixel), with scale bookkeeping to stay inside e4m3 range:
  - x arrives transposed in DR layout xt8[96, o=2, S] (d = 96*o + ki);
    q/k/v projections are single fp8 DR matmuls (weights pre-scaled x64 on
    host; q/k PSUM results cast straight to fp8 at x64, v cast with a 1/32
    scale so v8 = 2*v_true keeps the later oT->fp8 cast in range).
  - scoresT[k,q] = kT.T @ qT in the transposed layout so the exp() output
    (ACT, combined scale (1/sqrt(D))/4096 folds away the x64 q/k scales) is
    already the moving operand of the oT accumulation matmul.
  - A (1/32)-column appended to v makes the softmax row-sum fall out of the
    oT matmul as an extra row. oT accumulates as 96+97 partition split so
    the PSUM->fp8 DR-paired cast (oT8[97, o=2, 512], d = 96*o + ki) is
    partition-aligned; the epilogue output projection is then ONE fp8 DR
    matmul per 128-row tile against woe8[97, 2, 208] (Wo^T x64 plus a unit
    column that carries the row-sum), landing [q, e]-aligned for one
    reciprocal + fused scalar multiply (flags pre-divided by 4096 on host
    absorb all scales).
"""
import numpy as np
import ml_dtypes

import concourse.bacc as bacc
import concourse.tile as tile
import concourse.mybir as mybir
import concourse.bass_utils as _bu
from concourse.bass_utils import run_bass_kernel_spmd

# Extra walrus flags (e.g. --enable-double-pixel-opt) appended via env knob;
# BASSK_WALRUS_DEFAULT is always applied (double pixel measurably reduces
# PE stream time / PSUM port pressure on TRN2 for fp8 DR matmuls).
_WALRUS_DEFAULT = "--enable-double-pixel-opt"
if not getattr(_bu.get_walrus_args, "_extra_patched", False):
    _orig_gwa = _bu.get_walrus_args

    def _gwa(*a, **kw):
        import os as _os
        args = list(_orig_gwa(*a, **kw))
        args += _os.environ.get("BASSK_WALRUS_DEFAULT", _WALRUS_DEFAULT).split()
        extra = _os.environ.get("BASSK_WALRUS_EXTRA", "")
        if extra:
            args += extra.split()
        return args

    _gwa._extra_patched = True
    _bu.get_walrus_args = _gwa

B, S, D = 8, 4096, 192
MEM_READ, MEM_WRITE, MEM_READY = 156, 157, 158
P = 128          # partitions / tile rows
QB = 512         # q block (matmul free dim / PSUM bank)
NQB = S // QB    # 8
KC = 128         # key chunk (contraction tile)
NKC = S // KC    # 32
NT = S // P      # 32 row tiles
HD = 96          # half of D rounded to DR pairing (d = 96*o + ki)
SCALE = 1.0 / float(np.sqrt(D))
WS = 64.0        # host weight scale into fp8
C1 = 1.0 / 32.0  # ones-column value (row-sum scale)
CV = 1.0 / 64.0  # v cast scale: v8 = v_true (keeps |oT8| well under fp8 max)
KNUM = 2048.0    # accumulated numerator scale: (1*64)*(1/C1) = 2048
F32 = mybir.dt.float32
BF16 = mybir.dt.bfloat16
FP8 = mybir.dt.float8e4
DR = mybir.MatmulPerfMode.DoubleRow
VBLK = 208       # v_ext block stride (16B-aligned for DoubleRow lhsT step)
WOB = 208        # woe8 per-attention column block (16B-aligned DR step)
N_CORES = 8

_CACHE = {}


def _build():
    nc = bacc.Bacc("TRN2", target_bir_lowering=False, debug=False,
                   num_devices=N_CORES)
    x = nc.dram_tensor("x", [S, D], F32, kind="ExternalInput").ap()
    xt8 = nc.dram_tensor("xt8", [HD, 2 * S], FP8, kind="ExternalInput").ap()
    wqk8 = nc.dram_tensor("wqk8", [HD, 2 * 2 * 2 * 2 * HD], FP8,
                          kind="ExternalInput").ap()
    wv8 = nc.dram_tensor("wv8", [HD, 2 * 2 * D], FP8, kind="ExternalInput").ap()
    woe8 = nc.dram_tensor("woe8", [HD + 1, 2 * 2 * WOB], FP8,
                          kind="ExternalInput").ap()
    params = nc.dram_tensor("params", [P, 6], F32, kind="ExternalInput").ap()
    out = nc.dram_tensor("out", [S, D], F32, kind="ExternalOutput").ap()

    with tile.TileContext(nc) as tc:
        _emit(nc, tc, x, xt8, wqk8, wv8, woe8, params, out)
    nc.compile()
    return nc


def _emit(nc, tc, x, xt8, wqk8, wv8, woe8, params, out):
    from contextlib import ExitStack
    with ExitStack() as st:
        cpool = st.enter_context(tc.tile_pool(name="const", bufs=1))
        bigpool = st.enter_context(tc.tile_pool(name="big", bufs=1))
        apool = st.enter_context(tc.tile_pool(name="attn", bufs=6))
        opool = st.enter_context(tc.tile_pool(name="osb", bufs=2))
        xpool = st.enter_context(tc.tile_pool(name="xin", bufs=3))
        tpool = st.enter_context(tc.tile_pool(name="tmp", bufs=3))
        # PSUM budget (8 banks): mm 3x[128,1024]=6, oT0(96p)+oT1(97p) 1 bank
        # each; res tiles rotate through the oT0 slot (tag-shared)
        mmpool = st.enter_context(tc.tile_pool(name="mm", bufs=3, space="PSUM"))
        oaccpool = st.enter_context(tc.tile_pool(name="oacc", bufs=1, space="PSUM"))

        # resident constants / activations. Weights + params first (small,
        # gate everything); xt8 loads chunked so phase-A chunk ci only waits
        # for its own slice.
        pp = cpool.tile([P, 6], F32, tag="pp")
        nc.sync.dma_start(pp, params)
        wqk8s = cpool.tile([HD, 16 * HD], FP8, tag="wqk8s")
        nc.sync.dma_start(wqk8s, wqk8)
        wv8s = cpool.tile([HD, 4 * D], FP8, tag="wv8s")
        nc.sync.dma_start(wv8s, wv8)
        woe8s = cpool.tile([HD + 1, 4 * WOB], FP8, tag="woe8s")
        nc.sync.dma_start(woe8s, woe8)
        xt8s = cpool.tile([HD, 2 * S], FP8, tag="xt8s")
        for sb in range(NQB):
            for o in range(2):
                sl = slice(o * S + sb * QB, o * S + (sb + 1) * QB)
                nc.sync.dma_start(xt8s[:, sl], xt8[:, sl])
        # pre-fault the exp ACT table so the ~2.7us load overlaps input DMAs
        warm = cpool.tile([1, 1], F32, tag="warm")
        nc.scalar.activation(warm, pp[0:1, 0:1],
                             mybir.ActivationFunctionType.Exp)

        wqk6 = wqk8s.rearrange("p (o a j h e) -> p o a j h e",
                               o=2, a=2, j=2, h=2)
        wv4 = wv8s.rearrange("p (o a e) -> p o a e", o=2, a=2)
        woe4 = woe8s.rearrange("p (o a e) -> p o a e", o=2, a=2)
        xt3 = xt8s.rearrange("p (o s) -> p o s", o=2)

        # out accumulator [128, 32*192] f32 (tile g lives at cols g*192)
        out_acc = bigpool.tile([P, NT * D], F32, tag="out_acc")

        # per-attention persistent buffers (distinct tags so att1's phase A
        # can be emitted under att0's ACT-bound phase B)
        bufs = []
        for att in range(2):
            qTd = bigpool.tile([HD, 2 * S], FP8, tag=f"qTd{att}", name="qTd")
            kTd = bigpool.tile([HD, 2 * S], FP8, tag=f"kTd{att}", name="kTd")
            v_ext = bigpool.tile([P, NT * VBLK], FP8, tag=f"v_ext{att}",
                                 name="v_ext")
            bufs.append((qTd, kTd, v_ext))

        def phaseA_unit(att, ci, u):
            """Emit unit u (0..7) of phase-A chunk ci for `att`:
            u 0-1 = q halves, 2-3 = k halves, 4-7 = v tiles.
            qTd/kTd layout [96, 2, S] with e = 96*o + ki."""
            qTd, kTd, v_ext = bufs[att]
            if ci == 0 and u == 0:
                ones = v_ext.rearrange("p (t c) -> p t c", c=VBLK)[:, :, D:D + 1]
                nc.vector.memset(ones, C1)
            sb = ci
            if u < 4:
                dst = qTd if u < 2 else kTd
                j = u // 2
                h = u % 2
                ps = mmpool.tile([P, QB], F32, tag="mm", name="ps_proj")
                nc.tensor.matmul(
                    ps[:HD, :], wqk6[:, :, att, j, h, :],
                    xt3[:, :, sb * QB:(sb + 1) * QB],
                    start=True, stop=True, perf_mode=DR)
                nc.vector.tensor_copy(
                    dst[:, h * S + sb * QB:h * S + (sb + 1) * QB],
                    ps[:HD, :])
            else:
                t = 4 * ci + (u - 4)
                ps = mmpool.tile([P, QB], F32, tag="mm", name="ps_v")
                nc.tensor.matmul(ps[:, :D], xt3[:, :, t * P:(t + 1) * P],
                                 wv4[:, :, att, :],
                                 start=True, stop=True, perf_mode=DR)
                nc.vector.tensor_scalar(
                    v_ext[:, t * VBLK:t * VBLK + D], ps[:, :D],
                    pp[:, 4:5], None, op0=mybir.AluOpType.mult)

        def phaseA_chunk(att, ci):
            for u in range(8):
                phaseA_unit(att, ci, u)

        NPR = NKC // 2
        ostate = {}

        def phaseB_main(att, qb, interleave=None):
            qTd, kTd, v_ext = bufs[att]
            kT3 = kTd.rearrange("p (o s) -> p o s", o=2)
            qT3 = qTd.rearrange("p (o s) -> p o s", o=2)
            ve3 = v_ext.rearrange("p (t c) -> p t c", c=VBLK)
            # oT0 spans v cols 0:97 (97 rows) so the epilogue's DR-paired
            # fp8 cast fully covers oT8 plane 0 — row (ki=96, o=0) pairs a
            # zero row of woe8, but must hold FINITE data (fp8 garbage can
            # decode as NaN and NaN*0 poisons the matmul).
            oT0 = oaccpool.tile([HD + 1, QB], F32, tag="oT0")
            oT1 = oaccpool.tile([HD + 1, QB], F32, tag="oT1")
            ostate[(att, qb)] = (oT0, oT1)
            qs3 = qT3[:, :, qb * QB:(qb + 1) * QB]
            for pr in range(NPR):
                # two key-chunks' scoresT side by side in one 2-bank tile
                sc = mmpool.tile([P, 2 * QB], F32, tag="mm", name="sc")
                for h in range(2):
                    kc = 2 * pr + h
                    nc.tensor.matmul(sc[:, h * QB:(h + 1) * QB],
                                     kT3[:, :, kc * KC:(kc + 1) * KC],
                                     qs3, start=True, stop=True,
                                     perf_mode=DR)
                at = apool.tile([P, 2 * QB], FP8, tag="at")
                nc.scalar.activation(at, sc, mybir.ActivationFunctionType.Exp,
                                     scale=SCALE / (WS * WS))
                at3 = at.rearrange("p (o n) -> p o n", o=2)
                nc.tensor.matmul(oT0, ve3[:, 2 * pr:2 * pr + 2, 0:HD + 1],
                                 at3, start=(pr == 0), stop=(pr == NPR - 1),
                                 perf_mode=DR)
                nc.tensor.matmul(oT1, ve3[:, 2 * pr:2 * pr + 2, HD:D + 1],
                                 at3, start=(pr == 0), stop=(pr == NPR - 1),
                                 perf_mode=DR)
                if interleave is not None:
                    interleave(pr)

        def phaseB_epi(att, qb):
            flag_col = 1 + att
            oT0, oT1 = ostate.pop((att, qb))
            # PSUM -> fp8 DR-paired cast: oT8[97, o=2, 512], d = 96*o + ki.
            # Row (ki=96, o=0) is never written; woe8's matching row is 0.
            oT8 = opool.tile([HD + 1, 2 * QB], FP8, tag="oT8")
            o3 = oT8.rearrange("p (o n) -> p o n", o=2)
            nc.vector.tensor_copy(o3[:, 0, :], oT0)
            nc.vector.tensor_copy(o3[:, 1, :], oT1)

            for qt in range(4):
                g = qb * 4 + qt
                res_t = oaccpool.tile([P, QB], F32, tag="oT0", name="res")
                res = res_t[:, 0:WOB]
                nc.tensor.matmul(res, o3[:, :, qt * P:(qt + 1) * P],
                                 woe4[:, :, att, :],
                                 start=True, stop=True, perf_mode=DR)
                rec = tpool.tile([P, 1], F32, tag="rec")
                nc.vector.reciprocal(rec, res[:, D:D + 1])
                tmp = tpool.tile([P, D], F32, tag="tmp")
                nc.vector.tensor_scalar(
                    tmp, res[:, 0:D], rec, pp[:, flag_col:flag_col + 1],
                    op0=mybir.AluOpType.mult, op1=mybir.AluOpType.mult)
                acc = out_acc[:, g * D:(g + 1) * D]
                if att == 0:
                    xt = xpool.tile([P, D], F32, tag="xt")
                    nc.sync.dma_start(xt, x[g * P:(g + 1) * P, :])
                    nc.vector.tensor_scalar(
                        acc, xt, pp[:, 0:1], None, op0=mybir.AluOpType.mult)
                    nc.vector.tensor_add(acc, acc, tmp)
                else:
                    nc.vector.tensor_add(acc, acc, tmp)
                    nc.vector.memset(acc[:, MEM_READ:MEM_WRITE + 1], 0.0)
                    nc.vector.tensor_copy(acc[:, MEM_READY:MEM_READY + 1],
                                          pp[:, 3:4])
                    nc.sync.dma_start(out[g * P:(g + 1) * P, :], acc)

        # driver: A(0) units feed B(0,qb0) pair-by-pair (chunk ci complete
        # by pair 2ci); epilogues deferred one qb so the next qb's score
        # matmuls keep ACT fed; A(1) units spread across B(0)'s qb loops.
        def ilv0(pr):
            # head: only k/v units of att0 chunk ci = pr//2+1 (q for block
            # ci isn't needed until B(0,ci)); chunk1's q at pairs 14/15
            ci = pr // 2 + 1
            if ci < NQB:
                for u in ((2, 3, 4) if pr % 2 == 0 else (5, 6, 7)):
                    phaseA_unit(0, ci, u)
            elif pr in (14, 15):
                phaseA_unit(0, 1, pr - 14)

        def ilv_b0(qb):
            # under B(0,qb): att1 chunk qb-1 units at even pairs, att0
            # chunk qb+1's q units at pairs 1/3
            def f(pr):
                if pr % 2 == 0:
                    phaseA_unit(1, qb - 1, pr // 2)
                elif pr in (1, 3) and qb + 1 < NQB:
                    phaseA_unit(0, qb + 1, (pr - 1) // 2)
            return f

        def ilv_a1_last(pr):
            if pr % 2 == 0:
                phaseA_unit(1, NQB - 1, pr // 2)

        phaseA_chunk(0, 0)
        phaseB_main(0, 0, interleave=ilv0)
        for qb in range(1, NQB):
            phaseB_main(0, qb, interleave=ilv_b0(qb))
            phaseB_epi(0, qb - 1)
        for qb in range(NQB):
            # A(1) chunk 7 rides under B(1,0)'s first pairs
            ilv = ilv_a1_last if qb == 0 else None
            phaseB_main(1, qb, interleave=ilv)
            phaseB_epi(0 if qb == 0 else 1, NQB - 1 if qb == 0 else qb - 1)
        phaseB_epi(1, NQB - 1)


def _to_dr_layout(mat_t):
    """[192, N] (d-major) -> [96, 2, N] with d = 96*o + ki."""
    n = mat_t.shape[1]
    return np.ascontiguousarray(
        mat_t.reshape(2, HD, n).transpose(1, 0, 2))


def _prep_core_inputs(x_full, weights):
    """Host-side shard/layout prep. weights: dict of the 8 [192,192] f32."""
    f8 = ml_dtypes.float8_e4m3
    # q/k weights: wqk8[ki, o, a, j, h, eh] = WS * W[a][j][96h+eh, 96o+ki]
    wqk = np.zeros((HD, 2, 2, 2, 2, HD), np.float32)
    wv = np.zeros((HD, 2, 2, D), np.float32)
    woe = np.zeros((HD + 1, 2, 2, WOB), np.float32)
    for a, (nq, nk, nv, no) in enumerate(
            (("Wq_r", "Wk_r", "Wv_r", "Wo_r"),
             ("Wq_w", "Wk_w", "Wv_w", "Wo_w"))):
        for j, n in enumerate((nq, nk)):
            wt = _to_dr_layout(WS * weights[n].T)       # [96, 2, 192]
            wqk[:, :, a, j, :, :] = wt.reshape(HD, 2, 2, HD)
        wv[:, :, a, :] = _to_dr_layout(WS * weights[nv].T)
        woe[0:HD, :, a, 0:D] = _to_dr_layout(WS * weights[no].T)
        woe[HD, 1, a, D] = 1.0  # unit column carries the row-sum (d=192)
    in_maps = []
    for c in range(N_CORES):
        xb = np.ascontiguousarray(x_full[c]).astype(np.float32)  # [4096,192]
        xt = _to_dr_layout(np.ascontiguousarray(xb.T))           # [96,2,S]
        rg = float(xb[0, MEM_READ])
        wg = float(xb[0, MEM_WRITE])
        pvec = np.array([1.0 - rg - wg, rg / KNUM, wg / KNUM, rg + wg,
                         CV, 0.0], np.float32)
        in_maps.append({
            "x": xb,
            "xt8": xt.reshape(HD, 2 * S).astype(f8),
            "wqk8": wqk.reshape(HD, 16 * HD).astype(f8),
            "wv8": wv.reshape(HD, 4 * D).astype(f8),
            "woe8": woe.reshape(HD + 1, 4 * WOB).astype(f8),
            "params": np.tile(pvec, (P, 1)),
        })
    return in_maps


def _run(inputs, **spmd_kwargs):
    if "nc" not in _CACHE:
        _CACHE["nc"] = _build()
    nc = _CACHE["nc"]
    x_full = np.asarray(inputs["x"], np.float32)
    weights = {k: np.asarray(inputs[k], np.float32) for k in
               ("Wq_r", "Wk_r", "Wv_r", "Wo_r", "Wq_w", "Wk_w", "Wv_w", "Wo_w")}
    in_maps = _prep_core_inputs(x_full, weights)
    res = run_bass_kernel_spmd(nc, in_maps, list(range(N_CORES)), **spmd_kwargs)
    out = np.stack([res.results[c]["out"] for c in range(N_CORES)], axis=0)
    return out.astype(np.float32), res


def kernel(**inputs):
    out, _ = _run(inputs)
    return out


def kernel_traced(**inputs):
    """For test.py: also returns BassKernelResults with profile info."""
    return _run(inputs, trace=True)
